# revision 14
# baseline (speedup 1.0000x reference)
"""Trainium2 Bass kernel for nn_CapsuleLayer (capsule layer: einsum + squash).

  u_hat = einsum('croi,bri->bcro', W[0], x)   # x:[256,1152,8] W:[1,10,1152,16,8]
  out   = squash(u_hat)                       # squash over last (o) axis

Strategy (8 NeuronCores, routes sharded 144/core, full batch per core):
  - Per 4-route group: stationary = x^T block [32=(4 routes x 8 in), 128 batch],
    moving = block-diagonal weight block [32, 960] whose per-route 240 columns
    are [W_cr (16 cols per capsule) | L_cr (8 cols per capsule)], where
    L_cr = cholesky(W_cr^T W_cr).  The PE then produces both u (capsule
    outputs) and z = L^T x with ||z||^2 = ||u||^2, so the squash sq_norm
    reduce is over 8 instead of 16 elements.
  - squash scale s = sq/((1+sq)*sqrt(sq+1e-9)) = exp(0.5*ln(sq) - ln(1+sq))
    computed via ACT Ln/Exp (one table set; avoids banned Rsqrt/Reciprocal).
  - ACT squares z (PSUM->SBUF), DVE group-reduces, DVE broadcast-multiplies
    u by s straight out of PSUM into a dense SBUF tile, HWDGE DMA out.
"""

import sys

if "/opt/trn_rl_repo" not in sys.path:
    sys.path.insert(0, "/opt/trn_rl_repo")

from contextlib import ExitStack

import numpy as np

import concourse.bacc as bacc
import concourse.bass as bass
import concourse.mybir as mybir
import concourse.tile as tile
from concourse._compat import with_exitstack
from concourse.bass_utils import run_bass_kernel_spmd

# Problem shapes (hardcoded; harness provides full inputs)
B = 256          # batch
R = 1152         # num routes
C = 10           # num capsules
O = 16           # out channels
I = 8            # in channels
NCORES = 8
RL = R // NCORES                 # 144 routes per core
NT = RL // 8                     # 18 super-tile columns (8 routes each)
OZ = O + I                       # 24 cols per (route, capsule): 16 u + 8 z
RCOLS = C * OZ                   # 240 cols per route
F32 = mybir.dt.float32


@with_exitstack
def _capsule_body(ctx: ExitStack, tc: "tile.TileContext",
                  out: bass.AP, xs: bass.AP, wm: bass.AP, reps: int = 1):
    nc = tc.nc

    singles = ctx.enter_context(tc.tile_pool(name="singles", bufs=1))
    wm_pool = ctx.enter_context(tc.tile_pool(name="wm", bufs=NT))
    psum_pool = ctx.enter_context(tc.tile_pool(name="psum", bufs=2, space="PSUM"))
    zsq_pool = ctx.enter_context(tc.tile_pool(name="zsq", bufs=3))
    smalls = ctx.enter_context(tc.tile_pool(name="smalls", bufs=3))
    out_pool = ctx.enter_context(tc.tile_pool(name="outs", bufs=8))

    # Resident x stationaries: [64 rows, 18 super-tiles * 256 batch]
    xs_sb = singles.tile([64, NT * B], F32)
    nc.gpsimd.dma_start(out=xs_sb[:], in_=xs.rearrange("p t b -> p (t b)"))

    if reps > 1:
        # Timing-only variant: run the whole body `reps` times on-device so
        # wall-clock differences cancel host/axon overhead.
        loop_cm = tc.For_i(0, reps, 1)
        ctx.enter_context(loop_cm)

    for t in range(NT):
        wm_t = wm_pool.tile([64, 4 * RCOLS], F32)
        nc.gpsimd.dma_start(out=wm_t[:], in_=wm[t])
        for h in range(2):
            ps = psum_pool.tile([128, 4 * 512], F32)
            for s in range(2):
                lhsT = xs_sb[32 * s:32 * s + 32,
                             t * B + h * 128: t * B + h * 128 + 128]
                for q in range(2):
                    rhs = wm_t[32 * s:32 * s + 32, 480 * q:480 * q + 480]
                    k = 2 * s + q
                    nc.tensor.matmul(ps[:, 512 * k:512 * k + 480], lhsT, rhs,
                                     start=True, stop=True)
            # Strided views of psum: [128, k=4, r=2, c=10, v=24]
            pt = (ps[:].rearrange("p (k v) -> p k v", k=4)[:, :, 0:480]
                  .rearrange("p k (r c v) -> p k r c v", r=2, c=10))
            u_ap = pt[:, :, :, :, 0:O]
            z_ap = pt[:, :, :, :, O:OZ]

            zsq = zsq_pool.tile([128, 640], F32)
            nc.scalar.square(zsq[:], z_ap)

            sq = smalls.tile([128, 80], F32, tag="sq")
            nc.vector.tensor_reduce(
                out=sq[:], in_=zsq[:].rearrange("p (g v) -> p g v", v=I),
                axis=mybir.AxisListType.X, op=mybir.AluOpType.add)

            lnsq = smalls.tile([128, 80], F32, tag="lnsq")
            nc.scalar.activation(lnsq[:], sq[:], mybir.ActivationFunctionType.Ln)
            ln1p = smalls.tile([128, 80], F32, tag="ln1p")
            nc.scalar.activation(ln1p[:], sq[:], mybir.ActivationFunctionType.Ln,
                                 bias=1.0)
            w_t = smalls.tile([128, 80], F32, tag="w")
            # w = 0.5*ln(sq) - ln(1+sq)
            nc.vector.scalar_tensor_tensor(
                out=w_t[:], in0=lnsq[:], scalar=0.5, in1=ln1p[:],
                op0=mybir.AluOpType.mult, op1=mybir.AluOpType.subtract)
            s_t = smalls.tile([128, 80], F32, tag="s")
            nc.scalar.activation(s_t[:], w_t[:], mybir.ActivationFunctionType.Exp)

            s_b = (s_t[:].rearrange("p (k r c) -> p k r c", k=4, r=2)
                   .unsqueeze(4).broadcast_to([128, 4, 2, C, O]))
            ot = out_pool.tile([128, 4 * 2 * C * O], F32)
            nc.vector.tensor_mul(ot[:], u_ap, s_b)
            nc.sync.dma_start(out=out[h, t], in_=ot[:])


def build_bass(reps: int = 1):
    # Bacc (not plain Bass): its compile() runs generate_event_semaphores,
    # which splits multi-semaphore waits — TPB instructions carry only one
    # wait slot in hardware — plus move_matmul_waits_to_ldweights etc.
    nc = bacc.Bacc("TRN2", target_bir_lowering=False, debug=False,
                   num_devices=NCORES)
    xs = nc.dram_tensor("xs", [64, NT, B], F32, kind="ExternalInput")
    wm = nc.dram_tensor("wm", [NT, 64, 4 * RCOLS], F32, kind="ExternalInput")
    out = nc.dram_tensor("out", [2, NT, 128, 4 * 2 * C * O], F32,
                         kind="ExternalOutput")
    with tile.TileContext(nc) as tc:
        _capsule_body(tc, out[:], xs[:], wm[:], reps=reps)
    nc.compile()
    return nc


_NC = {}


def _get_nc(reps: int = 1):
    if reps not in _NC:
        _NC[reps] = build_bass(reps)
    return _NC[reps]


def _pack_inputs(x: np.ndarray, W: np.ndarray):
    """Build per-core xs [64, 18, 256] and wm [18, 64, 960] arrays."""
    x = np.ascontiguousarray(x, dtype=np.float32)
    W0 = np.ascontiguousarray(W.reshape(C, R, O, I), dtype=np.float32)

    # x stationaries: [R, I, B] -> per core [18, 2, 4, 8, 256] -> [64, 18, 256]
    xt = x.transpose(1, 2, 0)                       # [R, I, B]
    xs = xt.reshape(NCORES, NT, 2, 4, I, B)         # k, t, s, rl, i, b
    xs = xs.reshape(NCORES, NT, 64, B).transpose(0, 2, 1, 3)  # k, row, t, b
    xs = np.ascontiguousarray(xs)

    # Gram cholesky factors (fp64 for stability, cast to fp32)
    W64 = W0.astype(np.float64)
    G = np.einsum('croi,croj->crij', W64, W64)      # [C, R, I, I]
    L = np.linalg.cholesky(G)                       # [C, R, I, I] lower
    L32 = L.astype(np.float32)

    Wr = W0.transpose(1, 0, 2, 3)                   # [R, C, O, I]
    Lr = L32.transpose(1, 0, 2, 3)                  # [R, C, I, I] (i, k)

    Wrc = Wr.reshape(NCORES, NT, 2, 4, C, O, I)     # k,t,s,rl,c,o,i
    Lrc = Lr.reshape(NCORES, NT, 2, 4, C, I, I)     # k,t,s,rl,c,i,kk

    wm = np.zeros((NCORES, NT, 2, 4, I, 4, C, OZ), dtype=np.float32)
    for rl in range(4):
        # [k,t,s,c,o,i] -> [k,t,s,i,c,o]
        wm[:, :, :, rl, :, rl, :, 0:O] = Wrc[:, :, :, rl].transpose(0, 1, 2, 5, 3, 4)
        # [k,t,s,c,i,kk] -> [k,t,s,i,c,kk]
        wm[:, :, :, rl, :, rl, :, O:OZ] = Lrc[:, :, :, rl].transpose(0, 1, 2, 4, 3, 5)
    wm = wm.reshape(NCORES, NT, 64, 4 * RCOLS)
    return xs, wm


def _unpack_outputs(results):
    """Per-core out [2, NT, 128, 1280] -> full [B, C, R, O]."""
    full = np.empty((B, C, R, O), dtype=np.float32)
    for k in range(NCORES):
        ok = results[k]["out"].reshape(2, NT, 128, 2, 2, 2, C, O)
        # dims: h, t, p, s, q, r, c, o ; route_local = 8t + 4s + 2q + r
        fk = ok.transpose(0, 2, 6, 1, 3, 4, 5, 7).reshape(B, C, RL, O)
        full[:, :, k * RL:(k + 1) * RL, :] = fk
    return full


def run_packed(xs, wm, reps: int = 1):
    nc = _get_nc(reps)
    in_maps = [{"xs": xs[k], "wm": wm[k]} for k in range(NCORES)]
    return run_bass_kernel_spmd(nc, in_maps, list(range(NCORES)))


def kernel(x: np.ndarray, W: np.ndarray, **_ignored):
    x = np.asarray(x, dtype=np.float32)
    W = np.asarray(W, dtype=np.float32)
    assert x.shape == (B, R, I), x.shape
    xs, wm = _pack_inputs(x, W)
    res = run_packed(xs, wm)
    return _unpack_outputs(res.results)


# revision 16
# speedup vs baseline: 1.9436x; 1.9436x over previous
"""Trainium2 Bass kernel for nn_CapsuleLayer (capsule layer: einsum + squash).

  u_hat = einsum('croi,bri->bcro', W[0], x)   # x:[256,1152,8] W:[1,10,1152,16,8]
  out   = squash(u_hat)                       # squash over last (o) axis

Strategy (8 NeuronCores, routes sharded 144/core, full batch per core):
  - Per 4-route group: stationary = x^T block [32=(4 routes x 8 in), 128 batch],
    moving = block-diagonal weight block [32, 960] whose per-route 240 columns
    are [W_cr (16 cols per capsule) | L_cr (8 cols per capsule)], where
    L_cr = cholesky(W_cr^T W_cr).  The PE then produces both u (capsule
    outputs) and z = L^T x with ||z||^2 = ||u||^2, so the squash sq_norm
    reduce is over 8 instead of 16 elements.
  - squash scale s = sq/((1+sq)*sqrt(sq+1e-9)) = exp(0.5*ln(sq) - ln(1+sq))
    computed via ACT Ln/Exp (one table set; avoids banned Rsqrt/Reciprocal).
  - ACT squares z (PSUM->SBUF), DVE group-reduces, DVE broadcast-multiplies
    u by s straight out of PSUM into a dense SBUF tile, HWDGE DMA out.
"""

import sys

if "/opt/trn_rl_repo" not in sys.path:
    sys.path.insert(0, "/opt/trn_rl_repo")

from contextlib import ExitStack

import numpy as np

import concourse.bacc as bacc
import concourse.bass as bass
import concourse.mybir as mybir
import concourse.tile as tile
from concourse._compat import with_exitstack
from concourse.bass_utils import run_bass_kernel_spmd

# Problem shapes (hardcoded; harness provides full inputs)
B = 256          # batch
R = 1152         # num routes
C = 10           # num capsules
O = 16           # out channels
I = 8            # in channels
NCORES = 8
RL = R // NCORES                 # 144 routes per core
NT = RL // 8                     # 18 super-tile columns (8 routes each)
OZ = O + I                       # 24 cols per (route, capsule): 16 u + 8 z
RCOLS = C * OZ                   # 240 cols per route
F32 = mybir.dt.float32


@with_exitstack
def _capsule_body(ctx: ExitStack, tc: "tile.TileContext",
                  out: bass.AP, xs: bass.AP, wm: bass.AP, reps: int = 1):
    nc = tc.nc

    singles = ctx.enter_context(tc.tile_pool(name="singles", bufs=1))
    wm_pool = ctx.enter_context(tc.tile_pool(name="wm", bufs=NT))
    psum_pool = ctx.enter_context(tc.tile_pool(name="psum", bufs=2, space="PSUM"))
    zsq_pool = ctx.enter_context(tc.tile_pool(name="zsq", bufs=3))
    smalls = ctx.enter_context(tc.tile_pool(name="smalls", bufs=3))
    out_pool = ctx.enter_context(tc.tile_pool(name="outs", bufs=8))

    # Resident x stationaries: [64 rows, 18 super-tiles * 256 batch]
    xs_sb = singles.tile([64, NT * B], F32)
    nc.gpsimd.dma_start(out=xs_sb[:], in_=xs.rearrange("p t b -> p (t b)"))

    if reps > 1:
        # Timing-only variant: run the whole body `reps` times on-device so
        # wall-clock differences cancel host/axon overhead.
        loop_cm = tc.For_i(0, reps, 1)
        ctx.enter_context(loop_cm)

    for t in range(NT):
        wm_t = wm_pool.tile([64, 4 * RCOLS], F32)
        nc.gpsimd.dma_start(out=wm_t[:], in_=wm[t])
        for h in range(2):
            ps = psum_pool.tile([128, 4 * 512], F32)
            for s in range(2):
                lhsT = xs_sb[32 * s:32 * s + 32,
                             t * B + h * 128: t * B + h * 128 + 128]
                for q in range(2):
                    rhs = wm_t[32 * s:32 * s + 32, 480 * q:480 * q + 480]
                    k = 2 * s + q
                    nc.tensor.matmul(ps[:, 512 * k:512 * k + 480], lhsT, rhs,
                                     start=True, stop=True)
            # Strided views of psum: [128, k=4, r=2, c=10, v=24]
            pt = (ps[:].rearrange("p (k v) -> p k v", k=4)[:, :, 0:480]
                  .rearrange("p k (r c v) -> p k r c v", r=2, c=10))
            u_ap = pt[:, :, :, :, 0:O]
            z_ap = pt[:, :, :, :, O:OZ]

            zsq = zsq_pool.tile([128, 640], F32)
            nc.scalar.square(zsq[:], z_ap)

            sq = smalls.tile([128, 80], F32, tag="sq")
            nc.vector.tensor_reduce(
                out=sq[:], in_=zsq[:].rearrange("p (g v) -> p g v", v=I),
                axis=mybir.AxisListType.X, op=mybir.AluOpType.add)

            lnsq = smalls.tile([128, 80], F32, tag="lnsq")
            nc.scalar.activation(lnsq[:], sq[:], mybir.ActivationFunctionType.Ln)
            ln1p = smalls.tile([128, 80], F32, tag="ln1p")
            nc.scalar.activation(ln1p[:], sq[:], mybir.ActivationFunctionType.Ln,
                                 bias=1.0)
            w_t = smalls.tile([128, 80], F32, tag="w")
            # w = 0.5*ln(sq) - ln(1+sq)
            nc.vector.scalar_tensor_tensor(
                out=w_t[:], in0=lnsq[:], scalar=0.5, in1=ln1p[:],
                op0=mybir.AluOpType.mult, op1=mybir.AluOpType.subtract)
            s_t = smalls.tile([128, 80], F32, tag="s")
            nc.scalar.activation(s_t[:], w_t[:], mybir.ActivationFunctionType.Exp)

            s_b = (s_t[:].rearrange("p (k r c) -> p k r c", k=4, r=2)
                   .unsqueeze(4).broadcast_to([128, 4, 2, C, O]))
            ot = out_pool.tile([128, 4 * 2 * C * O], F32)
            nc.vector.tensor_mul(ot[:], u_ap, s_b)
            nc.sync.dma_start(out=out[h, t], in_=ot[:])


def build_bass(reps: int = 1):
    # Bacc (not plain Bass): its compile() runs generate_event_semaphores,
    # which splits multi-semaphore waits — TPB instructions carry only one
    # wait slot in hardware — plus move_matmul_waits_to_ldweights etc.
    nc = bacc.Bacc("TRN2", target_bir_lowering=False, debug=False,
                   num_devices=NCORES)
    xs = nc.dram_tensor("xs", [64, NT, B], F32, kind="ExternalInput")
    wm = nc.dram_tensor("wm", [NT, 64, 4 * RCOLS], F32, kind="ExternalInput")
    out = nc.dram_tensor("out", [2, NT, 128, 4 * 2 * C * O], F32,
                         kind="ExternalOutput")
    with tile.TileContext(nc) as tc:
        _capsule_body(tc, out[:], xs[:], wm[:], reps=reps)

    # All three ACT functions used here (Square, Ln, Exp) coexist in the
    # natural_log_exp_and_others table set, but the stock table-load pass
    # assigns each function its *first* containing set, alternating sets and
    # inserting ~2 table loads (~2.7us each) per super-tile.  Restricting the
    # candidate list to the one set that covers everything yields a single
    # load for the whole kernel (table loads are name-based pseudo-instrs).
    import types
    from concourse.hw_specs import get_activation_tables
    from concourse import bacc as _bacc_mod

    _PIN = "natural_log_exp_and_others"
    _FUNCS = {mybir.ActivationFunctionType.Square,
              mybir.ActivationFunctionType.Ln,
              mybir.ActivationFunctionType.Exp}

    def _one_set_table_loads(self):
        # act_func_set_id is positional, so keep the full list (ids aligned
        # with the compiler's act_info) and just remove our functions from
        # every other set so resolution lands on the pinned one.
        tables = [
            (k, (v if k == _PIN else (v - _FUNCS)))
            for k, v in get_activation_tables(self.m.arch).items()
        ]
        _bacc_mod._bass_rust.insert_act_table_loads(self, tables)

    nc.insert_act_table_loads = types.MethodType(_one_set_table_loads, nc)
    nc.compile()
    return nc


_NC = {}


def _get_nc(reps: int = 1):
    if reps not in _NC:
        _NC[reps] = build_bass(reps)
    return _NC[reps]


def _pack_inputs(x: np.ndarray, W: np.ndarray):
    """Build per-core xs [64, 18, 256] and wm [18, 64, 960] arrays."""
    x = np.ascontiguousarray(x, dtype=np.float32)
    W0 = np.ascontiguousarray(W.reshape(C, R, O, I), dtype=np.float32)

    # x stationaries: [R, I, B] -> per core [18, 2, 4, 8, 256] -> [64, 18, 256]
    xt = x.transpose(1, 2, 0)                       # [R, I, B]
    xs = xt.reshape(NCORES, NT, 2, 4, I, B)         # k, t, s, rl, i, b
    xs = xs.reshape(NCORES, NT, 64, B).transpose(0, 2, 1, 3)  # k, row, t, b
    xs = np.ascontiguousarray(xs)

    # Gram cholesky factors (fp64 for stability, cast to fp32)
    W64 = W0.astype(np.float64)
    G = np.einsum('croi,croj->crij', W64, W64)      # [C, R, I, I]
    L = np.linalg.cholesky(G)                       # [C, R, I, I] lower
    L32 = L.astype(np.float32)

    Wr = W0.transpose(1, 0, 2, 3)                   # [R, C, O, I]
    Lr = L32.transpose(1, 0, 2, 3)                  # [R, C, I, I] (i, k)

    Wrc = Wr.reshape(NCORES, NT, 2, 4, C, O, I)     # k,t,s,rl,c,o,i
    Lrc = Lr.reshape(NCORES, NT, 2, 4, C, I, I)     # k,t,s,rl,c,i,kk

    wm = np.zeros((NCORES, NT, 2, 4, I, 4, C, OZ), dtype=np.float32)
    for rl in range(4):
        # [k,t,s,c,o,i] -> [k,t,s,i,c,o]
        wm[:, :, :, rl, :, rl, :, 0:O] = Wrc[:, :, :, rl].transpose(0, 1, 2, 5, 3, 4)
        # [k,t,s,c,i,kk] -> [k,t,s,i,c,kk]
        wm[:, :, :, rl, :, rl, :, O:OZ] = Lrc[:, :, :, rl].transpose(0, 1, 2, 4, 3, 5)
    wm = wm.reshape(NCORES, NT, 64, 4 * RCOLS)
    return xs, wm


def _unpack_outputs(results):
    """Per-core out [2, NT, 128, 1280] -> full [B, C, R, O]."""
    full = np.empty((B, C, R, O), dtype=np.float32)
    for k in range(NCORES):
        ok = results[k]["out"].reshape(2, NT, 128, 2, 2, 2, C, O)
        # dims: h, t, p, s, q, r, c, o ; route_local = 8t + 4s + 2q + r
        fk = ok.transpose(0, 2, 6, 1, 3, 4, 5, 7).reshape(B, C, RL, O)
        full[:, :, k * RL:(k + 1) * RL, :] = fk
    return full


def run_packed(xs, wm, reps: int = 1):
    nc = _get_nc(reps)
    in_maps = [{"xs": xs[k], "wm": wm[k]} for k in range(NCORES)]
    return run_bass_kernel_spmd(nc, in_maps, list(range(NCORES)))


def kernel(x: np.ndarray, W: np.ndarray, **_ignored):
    x = np.asarray(x, dtype=np.float32)
    W = np.asarray(W, dtype=np.float32)
    assert x.shape == (B, R, I), x.shape
    xs, wm = _pack_inputs(x, W)
    res = run_packed(xs, wm)
    return _unpack_outputs(res.results)


# revision 53
# speedup vs baseline: 5.6301x; 2.8967x over previous
"""Trainium2 Bass kernel for nn_CapsuleLayer (capsule layer: einsum + squash).

  u_hat = einsum('croi,bri->bcro', W[0], x)   # x:[256,1152,8] W:[1,10,1152,16,8]
  out   = squash(u_hat)                       # squash over last (o) axis

Strategy (8 NeuronCores, routes sharded 144/core, full batch per core):
  - Groups of 3 routes.  Per (group, batch-half) ONE psum bank holds both:
      u-MM:  stationary x^T block [32=(3 routes x 8 in + pad), 128 batch],
             moving block-diagonal W [32, 480] -> psum[:, 0:480]
      sq-MM: stationary xx pair-products [128=(3 x 36 pairs + pad), 128 batch],
             moving block-diagonal sym-Gram cols [128, 30] -> psum[:, 480:510]
    where xx[b,(i,j)] = x_i*x_j (i<=j) and Gsym[(i,j),c] = (2-delta_ij)*G[i,j]
    with G = W_cr^T W_cr, so sq-MM emits sq_norm[b, (r,c)] = ||u||^2 directly.
    The PE therefore replaces both the ACT square pass and the DVE group
    reduce of a conventional squash implementation.
  - squash scale s = sq/((1+sq)*sqrt(sq+1e-9)) = exp(0.5*ln(sq) - ln(1+sq))
    via ACT Ln/Exp (single activation-table set; Rsqrt/Reciprocal on ACT are
    banned for accuracy and DVE reciprocal is slow).
  - DVE does only the final broadcast multiply u * s straight out of PSUM
    into dense SBUF tiles; HWDGE DMAs ship contiguous 240KB blocks.
  - Matmuls run in float32r (single-pass reduced-precision fp32, 4x faster
    than fp32's two half-speed passes; measured end-to-end error ~5e-4
    scale-relative vs the fp32 reference).
"""

import sys

if "/opt/trn_rl_repo" not in sys.path:
    sys.path.insert(0, "/opt/trn_rl_repo")

from contextlib import ExitStack

import numpy as np

import concourse.bacc as bacc
import concourse.bass as bass
import concourse.mybir as mybir
import concourse.tile as tile
from concourse._compat import with_exitstack
from concourse.bass_utils import run_bass_kernel_spmd

# Problem shapes (hardcoded; harness provides full inputs)
B = 256          # batch
R = 1152         # num routes
C = 10           # num capsules
O = 16           # out channels
I = 8            # in channels
NCORES = 8
RL = R // NCORES                 # 144 routes per core
NG = RL // 3                     # 48 groups of 3 routes
NQ = NG // 4                     # 12 quad-blocks of 4 groups (row strips)
NPAIR = 36                       # i<=j pairs of 8 inputs
F32 = mybir.dt.float32
PAIRS = [(i, j) for i in range(I) for j in range(i, I)]


@with_exitstack
def _capsule_body(ctx: ExitStack, tc: "tile.TileContext",
                  out: bass.AP, xs: bass.AP, wm: bass.AP,
                  xxs: bass.AP, gs: bass.AP, reps: int = 1,
                  mode: str = "full"):
    nc = tc.nc

    if "fp32" in mode:
        mm_dt = F32
    else:
        mm_dt = mybir.dt.float32r

    singles = ctx.enter_context(tc.tile_pool(name="singles", bufs=1))
    wm_pool = ctx.enter_context(tc.tile_pool(name="wm", bufs=4))
    xx_pool = ctx.enter_context(tc.tile_pool(name="xx", bufs=4))
    psum_pool = ctx.enter_context(tc.tile_pool(name="psum", bufs=8, space="PSUM"))
    smalls = ctx.enter_context(tc.tile_pool(name="smalls", bufs=4))
    out_pool = ctx.enter_context(tc.tile_pool(name="outs", bufs=8))

    # Resident stationaries / gram columns — full-128-partition DMAs (32- or
    # 64-partition transfers run at a fraction of DMA port bandwidth).
    xs_sb = singles.tile([128, NQ * B], mm_dt)
    nc.gpsimd.dma_start(out=xs_sb[:], in_=xs.rearrange("p q b -> p (q b)"))
    gs_sb = singles.tile([128, NG * 30], mm_dt)
    nc.gpsimd.dma_start(out=gs_sb[:], in_=gs.rearrange("p g n -> p (g n)"))

    if reps > 1:
        # Timing-only variant: run the whole body `reps` times on-device so
        # wall-clock differences cancel host/axon overhead.
        loop_cm = tc.For_i(0, reps, 1)
        ctx.enter_context(loop_cm)

    # Per q: 4 groups stacked on the 4 row strips (partition blocks of 32);
    # iterate two half-blocks of 2 groups x 2 halves = 4 psum banks, so the
    # scale chain runs once per half-block on [128, 120] while psum bufs=8
    # double-buffers half-blocks.
    for q in range(NQ):
        wm_t = wm_pool.tile([128, 480], mm_dt)
        nc.gpsimd.dma_start(out=wm_t[:], in_=wm[q])
        xx_t = xx_pool.tile([128, 4 * B], mm_dt)
        nc.gpsimd.dma_start(out=xx_t[:], in_=xxs[q].rearrange("p k b -> p (k b)"))
        for half in range(2):
            quads = []
            sqb = smalls.tile([128, 120], F32, tag="sqb")
            for kk in range(2):
                k = 2 * half + kk
                g = 4 * q + k
                for h in range(2):
                    ps = psum_pool.tile([128, 512], F32, tag="ps")
                    nc.tensor.matmul(
                        ps[:, 0:480],
                        xs_sb[32 * k:32 * k + 32,
                              q * B + h * 128: q * B + h * 128 + 128],
                        wm_t[32 * k:32 * k + 32, :], start=True, stop=True,
                        tile_position=(32 * k, 0))
                    nc.tensor.matmul(
                        ps[:, 480:510],
                        xx_t[:, k * B + h * 128: k * B + h * 128 + 128],
                        gs_sb[:, g * 30: g * 30 + 30], start=True, stop=True,
                        tile_position=(0, 0))
                    if "nosquash" in mode:
                        continue
                    j = 2 * kk + h
                    # sq: [128, (r3, c10)] -> copy into the block buffer (ACT
                    # is near PSUM and has slack; keeps DVE lean).
                    nc.scalar.copy(sqb[:, 30 * j: 30 * j + 30], ps[:, 480:510])
                    quads.append((ps, j, g, h))

            if "nosquash" in mode:
                continue

            # Scale chain once per half-block: s = exp(0.5*ln(sq) - ln(1+sq))
            lnsq = smalls.tile([128, 120], F32, tag="lnsq")
            nc.scalar.activation(lnsq[:], sqb[:],
                                 mybir.ActivationFunctionType.Ln)
            ln1p = smalls.tile([128, 120], F32, tag="ln1p")
            nc.scalar.activation(ln1p[:], sqb[:],
                                 mybir.ActivationFunctionType.Ln, bias=1.0)
            w_t = smalls.tile([128, 120], F32, tag="w")
            nc.vector.scalar_tensor_tensor(
                out=w_t[:], in0=lnsq[:], scalar=0.5, in1=ln1p[:],
                op0=mybir.AluOpType.mult, op1=mybir.AluOpType.subtract)
            s_t = smalls.tile([128, 120], F32, tag="s")
            nc.scalar.activation(s_t[:], w_t[:],
                                 mybir.ActivationFunctionType.Exp)

            for ps, j, g, h in quads:
                u_ap = (ps[:, 0:480]
                        .rearrange("p (r c v) -> p r c v", r=3, c=C))
                s_b = (s_t[:, 30 * j: 30 * j + 30]
                       .rearrange("p (r c) -> p r c", r=3)
                       .unsqueeze(3).broadcast_to([128, 3, C, O]))
                ot = out_pool.tile([128, 480], F32)
                nc.vector.tensor_mul(ot[:], u_ap, s_b)
                if "noout" not in mode:
                    nc.sync.dma_start(out=out[h, g], in_=ot[:])


def build_bass(reps: int = 1, mode: str = "full"):
    # Bacc (not plain Bass): its compile() runs generate_event_semaphores,
    # which splits multi-semaphore waits — TPB instructions carry only one
    # wait slot in hardware — plus move_matmul_waits_to_ldweights etc.
    nc = bacc.Bacc("TRN2", target_bir_lowering=False, debug=False,
                   num_devices=NCORES)
    in_dt = F32 if "fp32" in mode else mybir.dt.float32r
    xs = nc.dram_tensor("xs", [128, NQ, B], in_dt, kind="ExternalInput")
    wm = nc.dram_tensor("wm", [NQ, 128, 480], in_dt, kind="ExternalInput")
    xxs = nc.dram_tensor("xxs", [NQ, 128, 4, B], in_dt, kind="ExternalInput")
    gs = nc.dram_tensor("gs", [128, NG, 30], in_dt, kind="ExternalInput")
    out = nc.dram_tensor("out", [2, NG, 128, 480], F32, kind="ExternalOutput")
    with tile.TileContext(nc) as tc:
        _capsule_body(tc, out[:], xs[:], wm[:], xxs[:], gs[:],
                      reps=reps, mode=mode)

    # All ACT functions used here (Copy, Ln, Exp) coexist in the
    # natural_log_exp_and_others table set, but the stock table-load pass
    # assigns each function its *first* containing set, alternating sets and
    # inserting ~2.7us table loads throughout.  Strip our functions from all
    # other sets (keeping positional act_func_set ids intact) so resolution
    # lands on the one set and a single load is emitted.
    import types
    from concourse.hw_specs import get_activation_tables
    from concourse import bacc as _bacc_mod

    _PIN = "natural_log_exp_and_others"
    _FUNCS = {mybir.ActivationFunctionType.Square,
              mybir.ActivationFunctionType.Ln,
              mybir.ActivationFunctionType.Exp,
              mybir.ActivationFunctionType.Copy,
              mybir.ActivationFunctionType.Identity}

    def _one_set_table_loads(self):
        tables = [
            (k, (v if k == _PIN else (v - _FUNCS)))
            for k, v in get_activation_tables(self.m.arch).items()
        ]
        _bacc_mod._bass_rust.insert_act_table_loads(self, tables)

    nc.insert_act_table_loads = types.MethodType(_one_set_table_loads, nc)
    nc.compile()
    return nc


_NC = {}


def _get_nc(reps: int = 1, mode: str = "full"):
    key = (reps, mode)
    if key not in _NC:
        _NC[key] = build_bass(reps, mode)
    return _NC[key]


def _pack_inputs(x: np.ndarray, W: np.ndarray):
    """Build per-core xs [32,48,256], wm [48,32,480], xxs [48,128,256],
    gs [48,128,30]."""
    x = np.ascontiguousarray(x, dtype=np.float32)
    W0 = np.ascontiguousarray(W.reshape(C, R, O, I), dtype=np.float32)

    # x stationaries: [R, I, B] -> rows padded to 32, 4 groups stacked on the
    # 128 partitions (full-width DMA): [cores, 128=(k,row), NQ, B]
    xt = x.transpose(1, 2, 0)                        # [R, I, B]
    xs = np.zeros((NCORES, NG, 32, B), np.float32)
    xs[:, :, :24] = xt.reshape(NCORES, NG, 24, B)
    xs = xs.reshape(NCORES, NQ, 4, 32, B).transpose(0, 2, 3, 1, 4)
    xs = np.ascontiguousarray(xs.reshape(NCORES, 128, NQ, B))

    # W moving blocks, 4 groups stacked on partitions: [cores, NQ, 128, 480]
    Wt = W0.transpose(1, 3, 0, 2)                    # [R, I, C, O]
    Wt = Wt.reshape(NCORES, NG, 3, I, C * O)         # k,g,r,i,co
    wm = np.zeros((NCORES, NG, 32, 3, C * O), np.float32)
    for r in range(3):
        wm[:, :, r * I:(r + 1) * I, r] = Wt[:, :, r]
    wm = np.ascontiguousarray(wm.reshape(NCORES, NQ, 128, 480))

    # xx pair products: [B, R, 36] -> [cores, NQ, 4, (3*36 padded 128), B]
    ii = np.array([p[0] for p in PAIRS])
    jj = np.array([p[1] for p in PAIRS])
    xx = x[:, :, ii] * x[:, :, jj]                   # [B, R, 36]
    xxt = xx.transpose(1, 2, 0)                      # [R, 36, B]
    xxs = np.zeros((NCORES, NG, 128, B), np.float32)
    xxs[:, :, :108] = xxt.reshape(NCORES, NG, 108, B)
    xxs = np.ascontiguousarray(
        xxs.reshape(NCORES, NQ, 4, 128, B).transpose(0, 1, 3, 2, 4))

    # Gram columns: [cores, 48, 128, 30] block-diagonal over the 3 routes
    W64 = W0.astype(np.float64)
    G = np.einsum('croi,croj->crij', W64, W64)       # [C, R, I, I]
    Gsym = G[:, :, ii, jj] * np.where(ii == jj, 1.0, 2.0)   # [C, R, 36]
    Gt = Gsym.transpose(1, 2, 0).astype(np.float32)  # [R, 36, C]
    Gt = Gt.reshape(NCORES, NG, 3, NPAIR, C)
    gs = np.zeros((NCORES, NG, 128, 30), np.float32)
    for r in range(3):
        gs[:, :, r * NPAIR:(r + 1) * NPAIR, r * C:(r + 1) * C] = Gt[:, :, r]
    gs = np.ascontiguousarray(gs.transpose(0, 2, 1, 3))   # [cores, 128, 48, 30]
    return xs, wm, xxs, gs


def _unpack_outputs(results):
    """Per-core out [2, NG, 128, 480] -> full [B, C, R, O]."""
    full = np.empty((B, C, R, O), dtype=np.float32)
    for k in range(NCORES):
        ok = results[k]["out"].reshape(2, NG, 128, 3, C, O)
        # dims: h, g, p, r, c, o ; route_local = 3g + r
        fk = ok.transpose(0, 2, 4, 1, 3, 5).reshape(B, C, RL, O)
        full[:, :, k * RL:(k + 1) * RL, :] = fk
    return full


def run_packed(packed, reps: int = 1, mode: str = "full"):
    xs, wm, xxs, gs = packed
    nc = _get_nc(reps, mode)
    in_maps = [{"xs": xs[k], "wm": wm[k], "xxs": xxs[k], "gs": gs[k]}
               for k in range(NCORES)]
    return run_bass_kernel_spmd(nc, in_maps, list(range(NCORES)))


def kernel(x: np.ndarray, W: np.ndarray, **_ignored):
    x = np.asarray(x, dtype=np.float32)
    W = np.asarray(W, dtype=np.float32)
    assert x.shape == (B, R, I), x.shape
    packed = _pack_inputs(x, W)
    res = run_packed(packed)
    return _unpack_outputs(res.results)


# revision 64
# speedup vs baseline: 5.7566x; 1.0225x over previous
"""Trainium2 Bass kernel for nn_CapsuleLayer (capsule layer: einsum + squash).

  u_hat = einsum('croi,bri->bcro', W[0], x)   # x:[256,1152,8] W:[1,10,1152,16,8]
  out   = squash(u_hat)                       # squash over last (o) axis

Strategy (8 NeuronCores, routes sharded 144/core, full batch per core):
  - Groups of 3 routes.  Per (group, batch-half) ONE psum bank holds both:
      u-MM:  stationary x^T block [32=(3 routes x 8 in + pad), 128 batch],
             moving block-diagonal W [32, 480] -> psum[:, 0:480]
      sq-MM: stationary xx pair-products [128=(3 x 36 pairs + pad), 128 batch],
             moving block-diagonal sym-Gram cols [128, 30] -> psum[:, 480:510]
    where xx[b,(i,j)] = x_i*x_j (i<=j) and Gsym[(i,j),c] = (2-delta_ij)*G[i,j]
    with G = W_cr^T W_cr, so sq-MM emits sq_norm[b, (r,c)] = ||u||^2 directly.
    The PE therefore replaces both the ACT square pass and the DVE group
    reduce of a conventional squash implementation.
  - squash scale s = sq/((1+sq)*sqrt(sq+1e-9)) = exp(0.5*ln(sq) - ln(1+sq))
    via ACT Ln/Exp (single activation-table set; Rsqrt/Reciprocal on ACT are
    banned for accuracy and DVE reciprocal is slow).
  - DVE does only the final broadcast multiply u * s straight out of PSUM
    into dense SBUF tiles; HWDGE DMAs ship contiguous 240KB blocks.
  - Matmuls run in float32r (single-pass reduced-precision fp32, 4x faster
    than fp32's two half-speed passes; measured end-to-end error ~5e-4
    scale-relative vs the fp32 reference).
"""

import sys

if "/opt/trn_rl_repo" not in sys.path:
    sys.path.insert(0, "/opt/trn_rl_repo")

from contextlib import ExitStack

import numpy as np

import concourse.bacc as bacc
import concourse.bass as bass
import concourse.mybir as mybir
import concourse.tile as tile
from concourse._compat import with_exitstack
from concourse.bass_utils import run_bass_kernel_spmd

# Problem shapes (hardcoded; harness provides full inputs)
B = 256          # batch
R = 1152         # num routes
C = 10           # num capsules
O = 16           # out channels
I = 8            # in channels
NCORES = 8
RL = R // NCORES                 # 144 routes per core
NG = RL // 3                     # 48 groups of 3 routes
NQ = NG // 4                     # 12 quad-blocks of 4 groups (row strips)
NPAIR = 36                       # i<=j pairs of 8 inputs
F32 = mybir.dt.float32
PAIRS = [(i, j) for i in range(I) for j in range(i, I)]


@with_exitstack
def _capsule_body(ctx: ExitStack, tc: "tile.TileContext",
                  out: bass.AP, xs: bass.AP, wm: bass.AP,
                  xxs: bass.AP, gs: bass.AP, reps: int = 1,
                  mode: str = "full"):
    nc = tc.nc

    if "fp32" in mode:
        mm_dt = F32
    else:
        mm_dt = mybir.dt.float32r
    # Optional: sq-path operands (xx pair products + gram cols) in bf16 —
    # halves the largest input tensor and enables FWL on the sq-matmul
    # stationary load; costs ~2x on the scale accuracy.
    sq_dt = mybir.dt.bfloat16 if "bxx" in mode else mm_dt

    singles = ctx.enter_context(tc.tile_pool(name="singles", bufs=1))
    wm_pool = ctx.enter_context(tc.tile_pool(name="wm", bufs=4))
    xx_pool = ctx.enter_context(tc.tile_pool(name="xx", bufs=4))
    psum_pool = ctx.enter_context(tc.tile_pool(name="psum", bufs=8, space="PSUM"))
    smalls = ctx.enter_context(tc.tile_pool(name="smalls", bufs=4))
    out_pool = ctx.enter_context(tc.tile_pool(name="outs", bufs=8))

    # Resident stationaries / gram columns — full-128-partition DMAs (32- or
    # 64-partition transfers run at a fraction of DMA port bandwidth).
    xs_sb = singles.tile([128, NQ * B], mm_dt)
    nc.gpsimd.dma_start(out=xs_sb[:], in_=xs.rearrange("p q b -> p (q b)"))
    gs_sb = singles.tile([128, NG * 30], sq_dt)
    nc.gpsimd.dma_start(out=gs_sb[:], in_=gs.rearrange("p g n -> p (g n)"))

    if reps > 1:
        # Timing-only variant: run the whole body `reps` times on-device so
        # wall-clock differences cancel host/axon overhead.
        loop_cm = tc.For_i(0, reps, 1)
        ctx.enter_context(loop_cm)

    # Per q: 4 groups stacked on the 4 row strips (partition blocks of 32);
    # iterate two half-blocks of 2 groups x 2 halves = 4 psum banks, so the
    # scale chain runs once per half-block on [128, 120] while psum bufs=8
    # double-buffers half-blocks.
    for q in range(NQ):
        wm_t = wm_pool.tile([128, 480], mm_dt)
        nc.gpsimd.dma_start(out=wm_t[:], in_=wm[q])
        xx_t = xx_pool.tile([128, 4 * B], sq_dt)
        nc.gpsimd.dma_start(out=xx_t[:], in_=xxs[q].rearrange("p k b -> p (k b)"))
        for half in range(2):
            quads = []
            sqb = smalls.tile([128, 120], F32, tag="sqb")
            # All four strip-tiled u-MMs first (different row groups -> the
            # PE reorder window can pull their weight loads ahead), then the
            # four full-array sq-MMs.
            for kk in range(2):
                k = 2 * half + kk
                g = 4 * q + k
                for h in range(2):
                    ps = psum_pool.tile([128, 512], F32, tag="ps")
                    nc.tensor.matmul(
                        ps[:, 0:480],
                        xs_sb[32 * k:32 * k + 32,
                              q * B + h * 128: q * B + h * 128 + 128],
                        wm_t[32 * k:32 * k + 32, :], start=True, stop=True,
                        tile_position=(32 * k, 0))
                    quads.append((ps, 2 * kk + h, g, h))
            for ps, j, g, h in quads:
                k = g - 4 * q
                nc.tensor.matmul(
                    ps[:, 480:510],
                    xx_t[:, k * B + h * 128: k * B + h * 128 + 128],
                    gs_sb[:, g * 30: g * 30 + 30], start=True, stop=True,
                    tile_position=(0, 0))
                if "nosquash" not in mode:
                    # sq: [128, (r3, c10)] -> copy into the block buffer (ACT
                    # is near PSUM and has slack; keeps DVE lean).
                    nc.scalar.copy(sqb[:, 30 * j: 30 * j + 30], ps[:, 480:510])

            if "nosquash" in mode:
                continue

            # Scale chain once per half-block: s = exp(0.5*ln(sq) - ln(1+sq))
            lnsq = smalls.tile([128, 120], F32, tag="lnsq")
            nc.scalar.activation(lnsq[:], sqb[:],
                                 mybir.ActivationFunctionType.Ln)
            ln1p = smalls.tile([128, 120], F32, tag="ln1p")
            nc.scalar.activation(ln1p[:], sqb[:],
                                 mybir.ActivationFunctionType.Ln, bias=1.0)
            w_t = smalls.tile([128, 120], F32, tag="w")
            nc.vector.scalar_tensor_tensor(
                out=w_t[:], in0=lnsq[:], scalar=0.5, in1=ln1p[:],
                op0=mybir.AluOpType.mult, op1=mybir.AluOpType.subtract)
            s_t = smalls.tile([128, 120], F32, tag="s")
            nc.scalar.activation(s_t[:], w_t[:],
                                 mybir.ActivationFunctionType.Exp)

            # Merged output tiles: the two kk-groups for one h are adjacent
            # in DRAM (g, g+1), so two muls share one [128, 960] tile and a
            # single 491KB store (96 -> 48 output DMAs).
            g0 = 4 * q + 2 * half
            ots = {}
            for ps, j, g, h in quads:
                if h not in ots:
                    ot_new = out_pool.tile([128, 960], F32, tag="ot")
                    ots[h] = ot_new
                u_ap = (ps[:, 0:480]
                        .rearrange("p (r c v) -> p r c v", r=3, c=C))
                s_b = (s_t[:, 30 * j: 30 * j + 30]
                       .rearrange("p (r c) -> p r c", r=3)
                       .unsqueeze(3).broadcast_to([128, 3, C, O]))
                kk = g - g0
                nc.vector.tensor_mul(
                    ots[h][:, 480 * kk: 480 * kk + 480]
                    .rearrange("p (r c v) -> p r c v", r=3, c=C), u_ap, s_b)
            if "noout" not in mode:
                for h in range(2):
                    nc.sync.dma_start(
                        out=out[h, g0:g0 + 2].rearrange("g p v -> p g v"),
                        in_=ots[h][:].rearrange("p (g v) -> p g v", g=2))


def build_bass(reps: int = 1, mode: str = "full"):
    # Bacc (not plain Bass): its compile() runs generate_event_semaphores,
    # which splits multi-semaphore waits — TPB instructions carry only one
    # wait slot in hardware — plus move_matmul_waits_to_ldweights etc.
    nc = bacc.Bacc("TRN2", target_bir_lowering=False, debug=False,
                   num_devices=NCORES)
    in_dt = F32 if "fp32" in mode else mybir.dt.float32r
    sq_in_dt = mybir.dt.bfloat16 if "bxx" in mode else in_dt
    xs = nc.dram_tensor("xs", [128, NQ, B], in_dt, kind="ExternalInput")
    wm = nc.dram_tensor("wm", [NQ, 128, 480], in_dt, kind="ExternalInput")
    xxs = nc.dram_tensor("xxs", [NQ, 128, 4, B], sq_in_dt, kind="ExternalInput")
    gs = nc.dram_tensor("gs", [128, NG, 30], sq_in_dt, kind="ExternalInput")
    out = nc.dram_tensor("out", [2, NG, 128, 480], F32, kind="ExternalOutput")
    with tile.TileContext(nc) as tc:
        _capsule_body(tc, out[:], xs[:], wm[:], xxs[:], gs[:],
                      reps=reps, mode=mode)

    # All ACT functions used here (Copy, Ln, Exp) coexist in the
    # natural_log_exp_and_others table set, but the stock table-load pass
    # assigns each function its *first* containing set, alternating sets and
    # inserting ~2.7us table loads throughout.  Strip our functions from all
    # other sets (keeping positional act_func_set ids intact) so resolution
    # lands on the one set and a single load is emitted.
    import types
    from concourse.hw_specs import get_activation_tables
    from concourse import bacc as _bacc_mod

    _PIN = "natural_log_exp_and_others"
    _FUNCS = {mybir.ActivationFunctionType.Square,
              mybir.ActivationFunctionType.Ln,
              mybir.ActivationFunctionType.Exp,
              mybir.ActivationFunctionType.Copy,
              mybir.ActivationFunctionType.Identity}

    def _one_set_table_loads(self):
        tables = [
            (k, (v if k == _PIN else (v - _FUNCS)))
            for k, v in get_activation_tables(self.m.arch).items()
        ]
        _bacc_mod._bass_rust.insert_act_table_loads(self, tables)

    nc.insert_act_table_loads = types.MethodType(_one_set_table_loads, nc)
    nc.compile()
    return nc


_NC = {}


def _get_nc(reps: int = 1, mode: str = "full"):
    key = (reps, mode)
    if key not in _NC:
        _NC[key] = build_bass(reps, mode)
    return _NC[key]


def _pack_inputs(x: np.ndarray, W: np.ndarray):
    """Build per-core xs [32,48,256], wm [48,32,480], xxs [48,128,256],
    gs [48,128,30]."""
    x = np.ascontiguousarray(x, dtype=np.float32)
    W0 = np.ascontiguousarray(W.reshape(C, R, O, I), dtype=np.float32)

    # x stationaries: [R, I, B] -> rows padded to 32, 4 groups stacked on the
    # 128 partitions (full-width DMA): [cores, 128=(k,row), NQ, B]
    xt = x.transpose(1, 2, 0)                        # [R, I, B]
    xs = np.zeros((NCORES, NG, 32, B), np.float32)
    xs[:, :, :24] = xt.reshape(NCORES, NG, 24, B)
    xs = xs.reshape(NCORES, NQ, 4, 32, B).transpose(0, 2, 3, 1, 4)
    xs = np.ascontiguousarray(xs.reshape(NCORES, 128, NQ, B))

    # W moving blocks, 4 groups stacked on partitions: [cores, NQ, 128, 480]
    Wt = W0.transpose(1, 3, 0, 2)                    # [R, I, C, O]
    Wt = Wt.reshape(NCORES, NG, 3, I, C * O)         # k,g,r,i,co
    wm = np.zeros((NCORES, NG, 32, 3, C * O), np.float32)
    for r in range(3):
        wm[:, :, r * I:(r + 1) * I, r] = Wt[:, :, r]
    wm = np.ascontiguousarray(wm.reshape(NCORES, NQ, 128, 480))

    # xx pair products: [B, R, 36] -> [cores, NQ, 4, (3*36 padded 128), B]
    ii = np.array([p[0] for p in PAIRS])
    jj = np.array([p[1] for p in PAIRS])
    xx = x[:, :, ii] * x[:, :, jj]                   # [B, R, 36]
    xxt = xx.transpose(1, 2, 0)                      # [R, 36, B]
    xxs = np.zeros((NCORES, NG, 128, B), np.float32)
    xxs[:, :, :108] = xxt.reshape(NCORES, NG, 108, B)
    xxs = np.ascontiguousarray(
        xxs.reshape(NCORES, NQ, 4, 128, B).transpose(0, 1, 3, 2, 4))

    # Gram columns: [cores, 48, 128, 30] block-diagonal over the 3 routes
    W64 = W0.astype(np.float64)
    G = np.einsum('croi,croj->crij', W64, W64)       # [C, R, I, I]
    Gsym = G[:, :, ii, jj] * np.where(ii == jj, 1.0, 2.0)   # [C, R, 36]
    Gt = Gsym.transpose(1, 2, 0).astype(np.float32)  # [R, 36, C]
    Gt = Gt.reshape(NCORES, NG, 3, NPAIR, C)
    gs = np.zeros((NCORES, NG, 128, 30), np.float32)
    for r in range(3):
        gs[:, :, r * NPAIR:(r + 1) * NPAIR, r * C:(r + 1) * C] = Gt[:, :, r]
    gs = np.ascontiguousarray(gs.transpose(0, 2, 1, 3))   # [cores, 128, 48, 30]
    return xs, wm, xxs, gs


def _unpack_outputs(results):
    """Per-core out [2, NG, 128, 480] -> full [B, C, R, O]."""
    full = np.empty((B, C, R, O), dtype=np.float32)
    for k in range(NCORES):
        ok = results[k]["out"].reshape(2, NG, 128, 3, C, O)
        # dims: h, g, p, r, c, o ; route_local = 3g + r
        fk = ok.transpose(0, 2, 4, 1, 3, 5).reshape(B, C, RL, O)
        full[:, :, k * RL:(k + 1) * RL, :] = fk
    return full


def run_packed(packed, reps: int = 1, mode: str = "full"):
    xs, wm, xxs, gs = packed
    if "bxx" in mode:
        import ml_dtypes
        xxs = xxs.astype(ml_dtypes.bfloat16)
        gs = gs.astype(ml_dtypes.bfloat16)
    nc = _get_nc(reps, mode)
    in_maps = [{"xs": xs[k], "wm": wm[k], "xxs": xxs[k], "gs": gs[k]}
               for k in range(NCORES)]
    return run_bass_kernel_spmd(nc, in_maps, list(range(NCORES)))


def kernel(x: np.ndarray, W: np.ndarray, **_ignored):
    x = np.asarray(x, dtype=np.float32)
    W = np.asarray(W, dtype=np.float32)
    assert x.shape == (B, R, I), x.shape
    packed = _pack_inputs(x, W)
    res = run_packed(packed)
    return _unpack_outputs(res.results)
